# revision 23
# baseline (speedup 1.0000x reference)
"""Cosine-similarity multi-head attention on 8 TRN2 NeuronCores.

Problem: B=4, N=2048, E=1024, H=16, D=64.
Sharding: core c handles batch b=c//2 and head-group g=c%2 (8 heads, 512
model cols). Each core computes its heads' attention and a partial output
projection; the host sums the two partials per batch and adds the folded
output bias.

Device-side layout: everything is computed transposed.
  xT [E, N] (host pre-transposes) ->
  qT/kT = W.T @ xT   [m, n]  (heads on partitions, tokens on free dim)
  v    = xT.T @ Wv   [n, m]  (natural)
  S^T[j, i] = kn_j . qn_i    (keys on partitions)
  outT[d, i] = sum_j v[j, d] exp(S^T[j, i])  (+ row 64 = softmax denom via
                                              a ones column in v)
  yT[eo, n] = sum_m Wo[m, eo] outT[m, n]     (transposed, DMA'd out)

DMA throughput in this environment is descriptor-count-bound (~160ns per
descriptor, one descriptor per partition-contiguous run), so all DRAM
tensors use partition-major host layouts: every partition's whole payload
is one contiguous run (x: 128 descriptors of 32KB instead of 1024 of 4KB).
Constants are built on-device with memset/affine_select instead of DMA.
"""

import sys

sys.path.insert(0, "/opt/trn_rl_repo")

import numpy as np
import ml_dtypes

B, N, E, H = 4, 2048, 1024, 16
D = E // H           # 64
M_CORES = 8
HC = H // 2          # heads per core = 8
EC = E // 2          # model cols per core = 512
ET = E // 128        # 8 e-tiles
NT = N // 128        # 16 n-tiles
MT = EC // 128       # 4 m-tiles (head pairs)
JT = N // 128        # 16 key tiles
BF16 = ml_dtypes.bfloat16

_CACHE = {}


def build_nc(repeat=1, variant="full"):
    """Build + finalize the single-core Bass program (same on all cores).

    repeat>1 duplicates the whole computation serially inside one NEFF —
    used by the bench harness to measure per-iteration time above the
    ~100ms axon dispatch overhead."""
    key = ("nc", repeat, variant)
    if key in _CACHE:
        return _CACHE[key]
    import concourse.bass as bass  # noqa: F401
    from concourse import bacc
    import concourse.mybir as mybir
    import concourse.tile as tile
    from concourse.masks import make_identity
    from contextlib import ExitStack

    f32 = mybir.dt.float32
    bf16 = mybir.dt.bfloat16
    AF = mybir.ActivationFunctionType

    # Make Exp and Ln resolve to the combined natural_log_exp table set so
    # the act-table-load pass doesn't ping-pong between exp_and_others and
    # natural_log on every projection/attention transition. Positions in the
    # table list are load-bearing (index == act_func_set_id), so only the
    # function sets are filtered.
    if not getattr(bacc, "_act_tables_patched", False):
        _orig_gat = bacc.get_activation_tables

        def _gat(arch):
            t = dict(_orig_gat(arch))
            for k in t:
                if k != "natural_log_exp_and_others":
                    t[k] = {
                        f for f in t[k]
                        if str(f).split(".")[-1] not in ("Exp", "Ln")
                    }
            return t

        bacc.get_activation_tables = _gat
        bacc._act_tables_patched = True

    nc = bacc.Bacc()
    # All inputs partition-major: [128, ...] with everything one partition
    # needs contiguous along the trailing dims.
    xT = nc.declare_dram_parameter("xT", [128, ET, N], bf16, isOutput=False)
    wq = nc.declare_dram_parameter("wq", [128, ET, EC], bf16, isOutput=False)
    wk = nc.declare_dram_parameter("wk", [128, ET, EC], bf16, isOutput=False)
    wv = nc.declare_dram_parameter("wv", [128, ET, EC], bf16, isOutput=False)
    wo = nc.declare_dram_parameter("wo", [128, MT, E], bf16, isOutput=False)
    qkb8 = nc.declare_dram_parameter("qkb8", [8, 128], f32, isOutput=False)
    y = nc.declare_dram_parameter("y", [128, ET, N], bf16, isOutput=True)

    with tile.TileContext(nc) as tc:
      for _rep in range(repeat):
        with ExitStack() as ctx:
            cpool = ctx.enter_context(tc.sbuf_pool(name="consts", bufs=1))
            wqkv = ctx.enter_context(tc.sbuf_pool(name="wqkv", bufs=1))
            wop = ctx.enter_context(tc.sbuf_pool(name="wo", bufs=1))
            xp = ctx.enter_context(tc.sbuf_pool(name="xT", bufs=1))
            qkp = ctx.enter_context(tc.sbuf_pool(name="qkv", bufs=1))
            otp = ctx.enter_context(tc.sbuf_pool(name="outT", bufs=1))
            stg = ctx.enter_context(tc.sbuf_pool(name="stg", bufs=2))
            ep = ctx.enter_context(tc.sbuf_pool(name="exp", bufs=3))
            yp = ctx.enter_context(tc.sbuf_pool(name="y", bufs=2))
            # PSUM: pp 2x1 banks + s 2x2 banks + pvA 1 + pvB 1 = 8
            pp = ctx.enter_context(tc.psum_pool(name="pp", bufs=2))
            sp = ctx.enter_context(tc.psum_pool(name="sp", bufs=2))
            pvp = ctx.enter_context(tc.psum_pool(name="pvp", bufs=1))

            # ---- big input DMAs (one per tensor; 128 fat descriptors) ----
            xbig = xp.tile([128, ET * N], bf16, tag="xbig", name="xbig")
            nc.sync.dma_start(
                xbig[:].rearrange("p (e n) -> p e n", e=ET), xT[:, :, :]
            )
            x_t = [xbig[:, et * N:(et + 1) * N] for et in range(ET)]
            w_t = {}
            for nm, drh in (("q", wq), ("k", wk), ("v", wv)):
                wbig = wqkv.tile([128, ET * EC], bf16, tag=f"wb{nm}",
                                 name=f"wb{nm}")
                nc.scalar.dma_start(
                    wbig[:].rearrange("p (e n) -> p e n", e=ET), drh[:, :, :]
                )
                for et in range(ET):
                    w_t[nm, et] = wbig[:, et * EC:(et + 1) * EC]
            wobig = wop.tile([128, MT * E], bf16, tag="wob", name="wob")
            nc.scalar.dma_start(
                wobig[:].rearrange("p (m n) -> p m n", m=MT), wo[:, :, :]
            )
            wo_t = [wobig[:, mt * E:(mt + 1) * E] for mt in range(MT)]

            # ---- constants built on device ----
            # qkb8 [8, 128] -> [128, 8] via identity matmul transpose
            qkb_t = cpool.tile([128, 8], f32, tag="qkb", name="qkb")
            if variant in ("c1", "c2"):
                nc.vector.memset(qkb_t[:], 0.0)
            else:
                qkb8_t = cpool.tile([8, 128], f32, tag="qkb8", name="qkb8")
                nc.sync.dma_start(qkb8_t[:], qkb8[:, :])
                id8 = cpool.tile([8, 8], f32, tag="id8", name="id8")
                make_identity(nc, id8[:])
                qkb_ps = pp.tile([128, 8], f32, tag="pp", name="qkb_ps")
                nc.tensor.matmul(qkb_ps[:], lhsT=qkb8_t[:], rhs=id8[:],
                                 start=True, stop=True)
                nc.vector.tensor_copy(qkb_t[:], qkb_ps[:])
            # masks: per-parity column sums / broadcasts
            msum_t = cpool.tile([128, 2], bf16, tag="msum", name="msum")
            nc.vector.memset(msum_t[:], 0.0)
            nc.vector.memset(msum_t[0:64, 0:1], 1.0)
            nc.vector.memset(msum_t[64:128, 1:2], 1.0)
            # mbc[p, f] = 1 iff 0 <= f - 64p < 64 (partition-1-offset memsets
            # are rejected by the BIR verifier, so build via affine_select)
            mbc_t = cpool.tile([2, 128], bf16, tag="mbc", name="mbc")
            nc.vector.memset(mbc_t[:], 1.0)
            nc.gpsimd.affine_select(
                out=mbc_t[:], in_=mbc_t[:],
                compare_op=mybir.AluOpType.is_ge, fill=0.0,
                base=0, pattern=[[1, 128]], channel_multiplier=-64)
            nc.gpsimd.affine_select(
                out=mbc_t[:], in_=mbc_t[:],
                compare_op=mybir.AluOpType.is_ge, fill=0.0,
                base=63, pattern=[[-1, 128]], channel_multiplier=64)
            eps_t = cpool.tile([2, 1], f32, tag="eps", name="eps")
            nc.vector.memset(eps_t[:], 1e-12)

            # persistent activations
            qn_t = [qkp.tile([128, N], bf16, tag=f"qn{mt}", name=f"qn{mt}")
                    for mt in range(MT)]
            kn_t = [qkp.tile([128, N], bf16, tag=f"kn{mt}", name=f"kn{mt}")
                    for mt in range(MT)]
            v_t = [qkp.tile([128, HC * (D + 1)], bf16, tag=f"v{nt}",
                            name=f"v{nt}") for nt in range(NT)]
            outT_t = [otp.tile([128, N], bf16, tag=f"ot{mt}", name=f"ot{mt}")
                      for mt in range(MT)]

            # ---- V projection (natural layout [n, m]) + ones column ----
            # The 65th column of each head's v block makes the pv matmul
            # emit the softmax denominator as psum row 64 for free; concurrent
            # tiled matmuls share the moving-operand bus (no throughput
            # gain), so a separate ones-lhsT denominator matmul pair would
            # cost a full extra 1024 streamed columns per jt.
            for nt in range(0 if variant in ("dmaonly", "c1") else NT):
                ps = pp.tile([128, 512], f32, tag="pp", name="pp")
                for et in range(ET):
                    nc.tensor.matmul(
                        ps[:],
                        lhsT=x_t[et][:, nt * 128:(nt + 1) * 128],
                        rhs=w_t["v", et],
                        start=(et == 0),
                        stop=(et == ET - 1),
                    )
                vt = v_t[nt]
                v3 = vt[:].rearrange("p (h e) -> p h e", h=HC)
                nc.vector.tensor_copy(
                    v3[:, :, 0:D], ps[:].rearrange("p (h d) -> p h d", h=HC)
                )
                nc.vector.memset(v3[:, :, D:D + 1], 1.0)

            # ---- Q/K projections + l2 normalization ([m, n] layout) ----
            def qk_proj(mt, nm, dst, bias_col):
                """Generator: yields ~16x between PE chunks so the caller can
                interleave these instructions into the (exp-bound) attention
                stream of the previous head pair, keeping the PE busy enough
                that HAM never re-throttles its clock."""
                qf = stg.tile([128, N], f32, tag="qf", name="qf")
                for ch in range(4):
                    ps = pp.tile([128, 512], f32, tag="pp", name="pp")
                    for et in range(ET):
                        nc.tensor.matmul(
                            ps[:],
                            lhsT=w_t[nm, et][:, mt * 128:(mt + 1) * 128],
                            rhs=x_t[et][:, ch * 512:(ch + 1) * 512],
                            start=(et == 0),
                            stop=(et == ET - 1),
                        )
                        if et == 3:
                            yield
                    nc.vector.tensor_scalar_add(
                        qf[:, ch * 512:(ch + 1) * 512], ps[:],
                        qkb_t[:, bias_col:bias_col + 1],
                    )
                    yield
                rcp = stg.tile([2, N], bf16, tag="rcp", name="rcp", bufs=1)
                for ch in range(4):
                    sq = stg.tile([128, 512], bf16, tag="sq", name="sq")
                    nc.vector.tensor_mul(
                        sq[:], qf[:, ch * 512:(ch + 1) * 512],
                        qf[:, ch * 512:(ch + 1) * 512])
                    nps = pp.tile([2, 512], f32, tag="pp", name="npp")
                    nc.tensor.matmul(nps[:], lhsT=msum_t[:], rhs=sq[:],
                                     start=True, stop=True)
                    rs = stg.tile([2, 512], f32, tag="rs", name="rs")
                    # 1/sqrt(s+eps) = exp(-0.5*ln(s+eps)); Ln+Exp share one
                    # activation table set (sqrt's is separate and would
                    # thrash), and beat sqrt's 65536-ULP budget.
                    nc.scalar.activation(rs[:], nps[:], AF.Ln, bias=eps_t[:])
                    nc.scalar.activation(
                        rcp[:, ch * 512:(ch + 1) * 512], rs[:], AF.Exp,
                        scale=-0.5)
                    yield
                for ch in range(4):
                    bc = pp.tile([128, 512], f32, tag="pp", name="bc")
                    nc.tensor.matmul(
                        bc[:], lhsT=mbc_t[:],
                        rhs=rcp[:, ch * 512:(ch + 1) * 512],
                        start=True, stop=True,
                    )
                    nc.vector.tensor_mul(
                        dst[:, ch * 512:(ch + 1) * 512],
                        qf[:, ch * 512:(ch + 1) * 512], bc[:],
                    )
                    yield

            # ---- attention for one head pair (both heads of mt) ----
            # PE-array tiling gives 2x concurrency twice over:
            #   scores: head A on row-tile (0,0) [kn/qn partitions 0-63],
            #           head B on row-tile (64,0) -- K=64 each, run together.
            #   pv+den: head A on col-tile (0,0) [out partitions 0-63],
            #           head B on col-tile (0,64) -- M=64 each, run together,
            #           both in ONE [128,512] PSUM bank.
            # The softmax denominator is a separate ones-lhsT matmul pair
            # (also col-tiled, rows 0 and 64 of a pp-pool bank) instead of
            # the old 65th ones-column in v, which forced a 65-partition
            # (full-array) pv matmul. Per jt the PE now does ~640ns of work
            # vs 1.15us of ACT exp, so attention is exp-bound.
            def attend_pair(mt, bg=None):
                # bg: deque of generators of background PE work (next pair's
                # q/k projection, output-projection chunks) injected one
                # chunk per even jt to fill the PE's idle time under the
                # exp-bound attention stream.
                def inject():
                    while bg:
                        try:
                            next(bg[0])
                            return
                        except StopIteration:
                            bg.popleft()

                hA, hB = 2 * mt, 2 * mt + 1
                for ic4 in range(4):
                    i0 = ic4 * 512
                    pvA = pvp.tile([65, 512], f32, tag="pvA", name="pvA")
                    pvB = pvp.tile([65, 512], f32, tag="pvB", name="pvB")
                    for jt in range(JT):
                        if bg is not None and jt % 2 == 0:
                            inject()
                        s = sp.tile([128, 1024], f32, tag="s", name="s")
                        nc.tensor.matmul(
                            s[:, 0:512],
                            lhsT=kn_t[mt][0:64, jt * 128:(jt + 1) * 128],
                            rhs=qn_t[mt][0:64, i0:i0 + 512],
                            start=True, stop=True,
                        )
                        nc.tensor.matmul(
                            s[:, 512:1024],
                            lhsT=kn_t[mt][64:128, jt * 128:(jt + 1) * 128],
                            rhs=qn_t[mt][64:128, i0:i0 + 512],
                            start=True, stop=True,
                        )
                        e = ep.tile([128, 1024], bf16, tag="e", name="e")
                        if variant == "noexp":
                            nc.gpsimd.memset(e[:], 1.0)
                        else:
                            nc.scalar.activation(e[:], s[:], AF.Exp)
                        st, sp_ = (jt == 0), (jt == JT - 1)
                        nc.tensor.matmul(
                            pvA[:, :],
                            lhsT=v_t[jt][:, hA * (D + 1):(hA + 1) * (D + 1)],
                            rhs=e[:, 0:512], start=st, stop=sp_,
                        )
                        nc.tensor.matmul(
                            pvB[:, :],
                            lhsT=v_t[jt][:, hB * (D + 1):(hB + 1) * (D + 1)],
                            rhs=e[:, 512:1024], start=st, stop=sp_,
                        )
                    # Normalization entirely off the PE: stage the denom
                    # rows to SBUF (reciprocal_approx_fast requires
                    # base-partition-0 SBUF input), fast-approx reciprocal,
                    # partition-broadcast on the idle GpSimd engine,
                    # in-place DVE multiply.
                    dnA = stg.tile([1, 512], f32, tag="dnA", name="dnA",
                                   bufs=1)
                    dnB = stg.tile([1, 512], f32, tag="dnB", name="dnB",
                                   bufs=1)
                    nc.vector.tensor_copy(dnA[:], pvA[64:65, :])
                    nc.vector.tensor_copy(dnB[:], pvB[64:65, :])
                    nc.vector.tensor_copy(outT_t[mt][0:64, i0:i0 + 512],
                                          pvA[0:64, :])
                    nc.vector.tensor_copy(outT_t[mt][64:128, i0:i0 + 512],
                                          pvB[0:64, :])
                    rcA = stg.tile([1, 512], f32, tag="rcA", name="rcA",
                                   bufs=1)
                    rcB = stg.tile([1, 512], f32, tag="rcB", name="rcB",
                                   bufs=1)
                    nc.vector.reciprocal_approx_fast(rcA[:], dnA[:])
                    nc.vector.reciprocal_approx_fast(rcB[:], dnB[:])
                    bcA = stg.tile([128, 512], f32, tag="bcA", name="bcA",
                                   bufs=1)
                    bcB = stg.tile([128, 512], f32, tag="bcB", name="bcB",
                                   bufs=1)
                    nc.gpsimd.partition_broadcast(bcA[:], rcA[:],
                                                  channels=128)
                    nc.gpsimd.partition_broadcast(bcB[:], rcB[:],
                                                  channels=128)
                    nc.vector.tensor_mul(
                        outT_t[mt][0:64, i0:i0 + 512],
                        outT_t[mt][0:64, i0:i0 + 512], bcA[0:64, :])
                    nc.vector.tensor_mul(
                        outT_t[mt][64:128, i0:i0 + 512],
                        outT_t[mt][64:128, i0:i0 + 512], bcB[64:128, :])
                    if mt == MT - 1 and bg is not None:
                        # this 512-query column block is now fully
                        # normalized across all pairs: its slice of the
                        # output projection can run under the remaining
                        # attention stream.
                        bg.append(out_proj_cols(ic4))
                # drain leftover background work
                if bg is not None:
                    while bg:
                        for _ in bg.popleft():
                            pass

            # ---- output projection, transposed: yT[eo, n] ----
            # y tiles are bf16 (halves SBUF + DMA; the host assemble sums
            # the two per-batch partials in f32). Column-chunk ch covers
            # queries [512ch, 512ch+512) and only needs the normalized
            # outT columns of that range, so it can be injected under the
            # last pair's attention as soon as its ic4 chunk finishes.
            y_t = [yp.tile([128, N], bf16, tag=f"y{et}", name=f"y{et}",
                           bufs=1) for et in range(ET)]

            def out_proj_cols(ch):
                for et in range(ET):
                    ps = pp.tile([128, 512], f32, tag="pp", name="yps")
                    for mt in range(MT):
                        nc.tensor.matmul(
                            ps[:],
                            lhsT=wo_t[mt][:, et * 128:(et + 1) * 128],
                            rhs=outT_t[mt][:, ch * 512:(ch + 1) * 512],
                            start=(mt == 0), stop=(mt == MT - 1),
                        )
                    nc.vector.tensor_copy(y_t[et][:, ch * 512:(ch + 1) * 512],
                                          ps[:])
                    if ch == 3:
                        nc.sync.dma_start(y[:, et, :], y_t[et][:])
                    yield

            # interleave: projections of pair mt+1 and the output projection
            # run inside the (ACT-bound) attention streams so the PE never
            # idles long enough for HAM to re-throttle its clock.
            from collections import deque
            for mt in range(MT):
                if variant in ("dmaonly", "c1"):
                    for _ in out_proj_cols(3):
                        pass
                    break
                if mt == 0:
                    for _ in qk_proj(0, "q", qn_t[0], 0):
                        pass
                    for _ in qk_proj(0, "k", kn_t[0], 4):
                        pass
                bg = deque()
                if mt + 1 < MT:
                    bg.append(qk_proj(mt + 1, "q", qn_t[mt + 1], mt + 1))
                    bg.append(qk_proj(mt + 1, "k", kn_t[mt + 1], 4 + mt + 1))
                attend_pair(mt, bg)

    nc.finalize()
    _CACHE[key] = nc
    return nc


def make_in_maps(x, Wq_w, Wq_b, Wk_w, Wk_b, Wv_w, Wv_b, Wo_w, Wo_b):
    x = np.asarray(x, dtype=np.float32)

    def pmajor(a, tiles):
        # [tiles*128, F] -> [128, tiles, F] (partition-major)
        return np.ascontiguousarray(
            a.reshape(tiles, 128, a.shape[1]).transpose(1, 0, 2)
        ).astype(BF16)

    in_maps = []
    for c in range(M_CORES):
        b, g = c // 2, c % 2
        cols = slice(g * EC, (g + 1) * EC)
        qb = np.asarray(Wq_b, np.float32)[cols].reshape(MT, 128)
        kb = np.asarray(Wk_b, np.float32)[cols].reshape(MT, 128)
        qkb8 = np.zeros((8, 128), np.float32)
        qkb8[0:MT] = qb
        qkb8[4:4 + MT] = kb
        in_maps.append({
            "xT": pmajor(np.ascontiguousarray(x[b].T), ET),
            "wq": pmajor(np.asarray(Wq_w, np.float32)[:, cols], ET),
            "wk": pmajor(np.asarray(Wk_w, np.float32)[:, cols], ET),
            "wv": pmajor(np.asarray(Wv_w, np.float32)[:, cols], ET),
            "wo": pmajor(np.asarray(Wo_w, np.float32)[cols, :], MT),
            "qkb8": qkb8,
        })
    return in_maps


def assemble(results, Wv_b, Wo_w, Wo_b):
    bias_eff = (np.asarray(Wv_b, np.float32) @ np.asarray(Wo_w, np.float32)
                + np.asarray(Wo_b, np.float32))
    out = np.empty((B, N, E), np.float32)
    for b in range(B):
        # y is [128, ET, N] partition-major of yT [E, N] (bf16 partials)
        yT = (np.asarray(results[2 * b]["y"], np.float32)
              + np.asarray(results[2 * b + 1]["y"], np.float32))
        yT = yT.transpose(1, 0, 2).reshape(E, N)
        out[b] = yT.T + bias_eff
    return out


def kernel(x, Wq_w, Wq_b, Wk_w, Wk_b, Wv_w, Wv_b, Wo_w, Wo_b):
    from concourse.bass_utils import run_bass_kernel_spmd

    nc = build_nc()
    in_maps = make_in_maps(x, Wq_w, Wq_b, Wk_w, Wk_b, Wv_w, Wv_b, Wo_w, Wo_b)
    res = run_bass_kernel_spmd(nc, in_maps, list(range(M_CORES)))
    return assemble(res.results, Wv_b, Wo_w, Wo_b)

